# revision 6
# baseline (speedup 1.0000x reference)
"""NimbusLinear (VQ codebook) Trainium2 kernel.

Math: the reference's selection/threshold/sign/tree_des_mat/softmax/argmax
chain is exactly a depth-4 binary-tree threshold descent per (row, codeblock):
  node j at level l compares chosen[n, c*4+l] > thresholds[c*15+j]
  leaf index -> one-hot Encoded[n, c*16+k]
and the final einsum is a dense matmul out = Encoded @ lut_perm with
lut_perm[k*256+c, j] = lut[j, c, k].

Device strategy (8 cores, data-parallel over N rows, 512 rows/core, no
collectives):
  - encode: 15 exact-fp32 threshold compares + mux-tree descent + one-hot on
    DVE, n-sliced so the PE can start ~11us in.
  - matmul: lut split as fp8e4m3 hi + fp8e4m3 lo (residual); both passes are
    fp8 DoubleRow matmuls contracting 256 rows per instruction (one-hot
    Encoded is exact in fp8).  Contraction rows ck = (2*kp+d)*256 + cc*128 + p
    pair over d = k-parity within a cc half.
  - two cc passes: the cc0 pass closes each (j, m) PSUM tile immediately to a
    bf16 partial in SBUF (2-3 live banks instead of 8+, so PE order is free);
    the cc1 pass merges partial + PSUM -> bf16 out on DVE.  bf16 partials add
    ~3.3e-3 scale-relative error; total ~4e-3 vs the 2e-2 gate.
  - PE warmup: dummy DoubleRow matmuls on zeroed tiles keep the PE busy from
    ~1us so the p-state ramp (0.65/1.2GHz for the first 3us) is spent before
    real work arrives, and the cost model's ramp window never hits real mms.

PE cost: 1024 DoubleRow matmuls x 256 cycles ~= 109us; lut DMA 32MB fp8
~= 93us; both streams run continuously and overlap.
"""

import sys

sys.path.insert(0, "/opt/trn_rl_repo")

import numpy as np
import ml_dtypes

K = 16
DEPTH = 4
C = 256
IN_FEATURES = 4096
OUT_FEATURES = 4096
N_ROWS = 4096
NCORES = 8
NSH = N_ROWS // NCORES  # 512 rows per core
NCHUNK = NSH // 128  # 4 partition chunks of rows per core
JSLABS = OUT_FEATURES // 512  # 8 output column slabs
LUT_BUFS = 8  # in-flight lut slab-half tiles (8KB/partition each)
N_WARM = 100  # PE warmup dummy matmuls

_CACHED = {}


def _level_of_node(i):
    return int(np.floor(np.log2(i + 1)))


def _build_program():
    import concourse.bacc as bacc
    import concourse.mybir as mybir
    import concourse.tile as tile
    import concourse.bass as bass

    f32 = mybir.dt.float32
    bf16 = mybir.dt.bfloat16
    fp8 = mybir.dt.float8e4

    nc = bacc.Bacc("TRN2", target_bir_lowering=False, debug=False,
                   num_devices=NCORES)

    # inputs (per-core shapes)
    xg = nc.dram_tensor("xg", [2, DEPTH, 128, NSH], f32, kind="ExternalInput")
    th = nc.dram_tensor("th", [2, 128, 15], f32, kind="ExternalInput")
    # l8[j, h, cc, p, kp, d, jj] = fp8 of (hi if h==0 else lo) of
    #   lut_perm[(2*kp+d)*256 + cc*128 + p, j*512 + jj]
    l8 = nc.dram_tensor("l8", [JSLABS, 2, 2, 128, 8, 2, 512], fp8,
                        kind="ExternalInput")
    out = nc.dram_tensor("out", [NCHUNK, 128, JSLABS, 512], bf16,
                         kind="ExternalOutput")

    gt = mybir.AluOpType.is_gt
    eq = mybir.AluOpType.is_equal
    add = mybir.AluOpType.add
    DR = mybir.MatmulPerfMode.DoubleRow

    with tile.TileContext(nc) as tc:
        # keep every pool open for the whole program: early closes let later
        # pools recycle SBUF ranges and inherit WAR waits on whole phases.
        with tc.tile_pool(name="enc", bufs=1) as encp, \
             tc.tile_pool(name="encwork", bufs=1) as wp, \
             tc.tile_pool(name="enctmp", bufs=1) as tp, \
             tc.tile_pool(name="lut", bufs=LUT_BUFS) as lutp, \
             tc.tile_pool(name="part", bufs=1) as pp, \
             tc.tile_pool(name="psum", bufs=8,
                          space=bass.MemorySpace.PSUM) as psp:

            # ---------------- PE warmup -----------------------------------
            wz = wp.tile([128, 2, 128], fp8, tag="wz")
            mz = wp.tile([128, 2, 512], fp8, tag="mz")
            nc.vector.memset(wz[:], 0.0)
            nc.vector.memset(mz[:], 0.0)
            pz = psp.tile([128, 512], f32, tag="ps", name="warm")
            for i in range(N_WARM):
                nc.tensor.matmul(pz[:], wz[:], mz[:],
                                 start=(i == 0), stop=(i == N_WARM - 1),
                                 perf_mode=DR)

            # ---------------- input DMAs (issue order matters) -------------
            tht = []
            xt = []
            for cc in range(2):
                t = wp.tile([128, 15], f32, tag=f"th{cc}")
                nc.sync.dma_start(t[:], th[cc])
                tht.append(t)
                row = []
                for l in range(DEPTH):
                    x = wp.tile([128, NSH], f32, tag=f"x{l}_{cc}",
                                name=f"x{l}_{cc}")
                    nc.sync.dma_start(x[:], xg[cc, l])
                    row.append(x)
                xt.append(row)

            # lut slab-half tiles, streamed cc0 j0..7 then cc1 j0..7
            lt = {}
            for cc in range(2):
                for j in range(JSLABS):
                    for h in range(2):
                        t = lutp.tile([128, 8, 2, 512], fp8, tag="lut",
                                      name=f"l{j}_{h}_{cc}")
                        nc.sync.dma_start(t[:], l8[j, h, cc])
                        lt[(j, h, cc)] = t

            # one-hot tiles. cc0/s0 is built in two 128-wide chains (m0, m1)
            # to cut the PE-start latency; the rest in 256-wide chains.
            # enc[(cc, piece, kp)]; pieces cover n-ranges listed in `pieces`.
            pieces = [(0, 0, 128), (0, 128, 128), (0, 256, 256),
                      (1, 0, 256), (1, 256, 256)]
            enc8 = {}
            for cc, off, w in pieces:
                for kp in range(8):
                    enc8[(cc, off, kp)] = encp.tile(
                        [128, 2, w], fp8, tag=f"e{cc}_{off}_{kp}",
                        name=f"e{cc}_{off}_{kp}")

            def encode_piece(cc, off, w):
                nsl = slice(off, off + w)
                B = [tp.tile([128, 256], bf16, tag=f"b{i}",
                             name=f"b{i}_{cc}{off}")[:, :w]
                     for i in range(15)]
                for i in range(15):
                    nc.vector.tensor_single_scalar(
                        B[i], xt[cc][_level_of_node(i)][:, nsl],
                        tht[cc][:, i:i + 1], gt)

                def mux(u, v, sel, tag):
                    # u + sel*(v-u), all values in {0,1} (exact in bf16)
                    t = tp.tile([128, 256], bf16, tag=tag,
                                name=f"mux_{tag}_{cc}{off}")[:, :w]
                    nc.vector.tensor_sub(t, v, u)
                    nc.vector.tensor_mul(t, t, sel)
                    nc.vector.tensor_add(t, t, u)
                    return t

                b0 = B[0]
                b1 = mux(B[1], B[2], b0, "m1")
                m0 = mux(B[3], B[4], b1, "m20")
                m1 = mux(B[5], B[6], b1, "m21")
                b2 = mux(m0, m1, b0, "m2")
                c00 = mux(B[7], B[8], b2, "c00")
                c01 = mux(B[9], B[10], b2, "c01")
                c10 = mux(B[11], B[12], b2, "c10")
                c11 = mux(B[13], B[14], b2, "c11")
                d0 = mux(c00, c01, b1, "d0")
                d1 = mux(c10, c11, b1, "d1")
                b3 = mux(d0, d1, b0, "d")

                # idx = 8*b0 + 4*b1 + 2*b2 + b3 (small ints, exact in bf16)
                idx = tp.tile([128, 256], bf16, tag="idx",
                              name=f"idx{cc}{off}")[:, :w]
                nc.vector.tensor_scalar_mul(idx, b0, 2.0)
                nc.vector.tensor_add(idx, idx, b1)
                nc.vector.tensor_scalar_mul(idx, idx, 2.0)
                nc.vector.tensor_add(idx, idx, b2)
                nc.vector.tensor_scalar_mul(idx, idx, 2.0)
                nc.vector.tensor_add(idx, idx, b3)

                for k in range(K):
                    nc.vector.tensor_single_scalar(
                        enc8[(cc, off, k // 2)][:, k % 2, :], idx,
                        float(k), eq)

            for cc, off, w in pieces:
                encode_piece(cc, off, w)

            # weight slice for (cc, m): the enc piece covering m's n-range
            def wslice(cc, m, kp):
                for pcc, off, w in pieces:
                    if pcc == cc and off <= m * 128 < off + w:
                        o = m * 128 - off
                        return enc8[(cc, off, kp)][:, :, o:o + 128]
                raise KeyError

            # ---------------- matmul passes --------------------------------
            # bf16 partials for the cc0 pass; merged in place at cc1 close.
            part = {}
            for cc in range(2):
                for j in range(JSLABS):
                    for m in range(NCHUNK):
                        ps = psp.tile([128, 512], f32, tag="ps",
                                      name=f"ps{cc}_{j}_{m}")
                        for kp in range(8):
                            w = wslice(cc, m, kp)
                            for h in range(2):
                                nc.tensor.matmul(
                                    ps[:], w, lt[(j, h, cc)][:, kp, :, :],
                                    start=(kp == 0 and h == 0),
                                    stop=(kp == 7 and h == 1),
                                    perf_mode=DR)
                        if cc == 0:
                            pt = pp.tile([128, 512], bf16, tag=f"pt{j}_{m}",
                                         name=f"pt{j}_{m}")
                            part[(j, m)] = pt
                            nc.scalar.copy(pt[:], ps[:])
                        else:
                            pt = part[(j, m)]
                            nc.vector.tensor_tensor(pt[:], pt[:], ps[:], add)
                            nc.sync.dma_start(out[m, :, j], pt[:])

    nc.compile()
    return nc


_BASE_TREE = np.array([
    [-1,-1,0,-1,0,0,0,-1,0,0,0,0,0,0,0],[-1,-1,0,-1,0,0,0,1,0,0,0,0,0,0,0],
    [-1,-1,0,1,0,0,0,0,-1,0,0,0,0,0,0],[-1,-1,0,1,0,0,0,0,1,0,0,0,0,0,0],
    [-1,1,0,0,-1,0,0,0,0,-1,0,0,0,0,0],[-1,1,0,0,-1,0,0,0,0,1,0,0,0,0,0],
    [-1,1,0,0,1,0,0,0,0,0,-1,0,0,0,0],[-1,1,0,0,1,0,0,0,0,0,1,0,0,0,0],
    [1,0,-1,0,0,-1,0,0,0,0,0,-1,0,0,0],[1,0,-1,0,0,-1,0,0,0,0,0,1,0,0,0],
    [1,0,-1,0,0,1,0,0,0,0,0,0,-1,0,0],[1,0,-1,0,0,1,0,0,0,0,0,0,1,0,0],
    [1,0,1,0,0,0,-1,0,0,0,0,0,0,-1,0],[1,0,1,0,0,0,-1,0,0,0,0,0,0,1,0],
    [1,0,1,0,0,0,1,0,0,0,0,0,0,0,-1],[1,0,1,0,0,0,1,0,0,0,0,0,0,0,1]],
    dtype=np.float32)


def _reference_structure_ok(selection_matrix, tree_des_mat):
    sm = np.asarray(selection_matrix)
    td = np.asarray(tree_des_mat)
    if sm.shape != (C * (K - 1), C * DEPTH) or td.shape != (C * K, C * (K - 1)):
        return False
    base_sel = np.zeros((K - 1, DEPTH), dtype=np.float32)
    base_sel[0, 0] = 1.0
    for i in range(1, K - 1):
        base_sel[i, int(np.log2(i + 1))] = 1.0
    exp_sm = np.zeros_like(sm)
    exp_td = np.ones_like(td)
    for i in range(C):
        exp_sm[i * (K - 1):(i + 1) * (K - 1), i * DEPTH:(i + 1) * DEPTH] = base_sel
        exp_td[i * K:(i + 1) * K, i * (K - 1):(i + 1) * (K - 1)] = _BASE_TREE
    return np.array_equal(sm, exp_sm) and np.array_equal(td, exp_td)


def _numpy_fallback(inputMatrix, dims, selection_matrix, thresholds,
                    tree_des_mat, lut):
    """Faithful numpy replication of the reference forward pass (slow)."""
    x = np.asarray(inputMatrix, np.float32)
    n = x.shape[0]
    c = lut.shape[1]
    chosen = x[:, np.asarray(dims).astype(np.int64)]
    subtracted = (np.asarray(selection_matrix, np.float32) @ chosen.T
                  - np.asarray(thresholds, np.float32))
    sign = np.sign(subtracted).astype(np.float32)
    tree_result = (np.asarray(tree_des_mat, np.float32) @ sign).T.reshape(n, c, K)
    index = np.argmax(tree_result, axis=2)
    onehot = np.eye(K, dtype=np.float32)[index]  # (n, c, K)
    lutm = np.asarray(lut, np.float32).transpose(1, 2, 0).reshape(c * K, -1)
    return (onehot.reshape(n, c * K) @ lutm).astype(np.float32)


def kernel(inputMatrix, dims, selection_matrix, thresholds, tree_des_mat, lut):
    inputMatrix = np.ascontiguousarray(np.asarray(inputMatrix, dtype=np.float32))
    dims_i = np.asarray(dims).astype(np.int64)
    thresholds = np.asarray(thresholds, dtype=np.float32)
    lut = np.asarray(lut, dtype=np.float32)

    if not _reference_structure_ok(selection_matrix, tree_des_mat):
        return _numpy_fallback(inputMatrix, dims_i, selection_matrix,
                               thresholds, tree_des_mat, lut)

    # ---- host prep ----
    chosen = inputMatrix[:, dims_i]  # (N, C*DEPTH)
    th3 = np.ascontiguousarray(thresholds.reshape(C, K - 1).reshape(2, 128, 15))

    # lut_perm[k*256+c, j] = lut[j, c, k]; fp8 hi + fp8 lo residual
    lut_perm = np.ascontiguousarray(
        lut.transpose(2, 1, 0).reshape(C * K, OUT_FEATURES))
    lut_hi = lut_perm.astype(ml_dtypes.float8_e4m3)
    lut_lo = (lut_perm - lut_hi.astype(np.float32)).astype(ml_dtypes.float8_e4m3)

    def dev_layout(a):
        # (4096 ck, 4096 j) -> [j, cc, p, kp, d, jj], ck = (2kp+d)*256+cc*128+p
        return a.reshape(8, 2, 2, 128, JSLABS, 512).transpose(4, 2, 3, 0, 1, 5)

    l8_np = np.ascontiguousarray(
        np.stack([dev_layout(lut_hi), dev_layout(lut_lo)], axis=1))

    from concourse.bass_utils import run_bass_kernel_spmd

    if "nc" not in _CACHED:
        _CACHED["nc"] = _build_program()
    nc = _CACHED["nc"]

    in_maps = []
    for g in range(NCORES):
        ch = chosen[g * NSH:(g + 1) * NSH].reshape(NSH, 2, 128, DEPTH)
        xg_np = np.ascontiguousarray(ch.transpose(1, 3, 2, 0))  # [cc, l, p, n]
        in_maps.append({"xg": xg_np, "th": th3, "l8": l8_np})

    res = run_bass_kernel_spmd(nc, in_maps, list(range(NCORES)))
    out = np.concatenate(
        [np.asarray(res.results[g]["out"]).astype(np.float32)
         .reshape(NSH, OUT_FEATURES) for g in range(NCORES)], axis=0)
    return out


# revision 10
# speedup vs baseline: 1.1054x; 1.1054x over previous
"""NimbusLinear (VQ codebook) Trainium2 kernel.

Math: the reference's selection/threshold/sign/tree_des_mat/softmax/argmax
chain is exactly a depth-4 binary-tree threshold descent per (row, codeblock):
  node j at level l compares chosen[n, c*4+l] > thresholds[c*15+j]
  leaf index -> one-hot Encoded[n, c*16+k]
and the final einsum is a dense matmul out = Encoded @ lut_perm with
lut_perm[k*256+c, j] = lut[j, c, k].

Device strategy (8 cores, data-parallel over N rows, 512 rows/core, no
collectives):
  - encode: 15 exact-fp32 threshold compares + mux-tree descent + one-hot on
    DVE, n-sliced so the PE can start ~11us in.
  - matmul: lut split as fp8e4m3 hi + fp8e4m3 lo (residual); both passes are
    fp8 DoubleRow matmuls contracting 256 rows per instruction (one-hot
    Encoded is exact in fp8).  Contraction rows ck = (2*kp+d)*256 + cc*128 + p
    pair over d = k-parity within a cc half.
  - two cc passes: the cc0 pass closes each (j, m) PSUM tile immediately to a
    bf16 partial in SBUF (2-3 live banks instead of 8+, so PE order is free);
    the cc1 pass merges partial + PSUM -> bf16 out on DVE.  bf16 partials add
    ~3.3e-3 scale-relative error; total ~4e-3 vs the 2e-2 gate.
  - PE warmup: dummy DoubleRow matmuls on zeroed tiles keep the PE busy from
    ~1us so the p-state ramp (0.65/1.2GHz for the first 3us) is spent before
    real work arrives, and the cost model's ramp window never hits real mms.

PE cost: 1024 DoubleRow matmuls x 256 cycles ~= 109us; lut DMA 32MB fp8
~= 93us; both streams run continuously and overlap.
"""

import sys

sys.path.insert(0, "/opt/trn_rl_repo")

import numpy as np
import ml_dtypes

K = 16
DEPTH = 4
C = 256
IN_FEATURES = 4096
OUT_FEATURES = 4096
N_ROWS = 4096
NCORES = 8
NSH = N_ROWS // NCORES  # 512 rows per core
NCHUNK = NSH // 128  # 4 partition chunks of rows per core
JSLABS = OUT_FEATURES // 512  # 8 output column slabs
LUT_BUFS = 14  # in-flight lut slab-half tiles (8KB/partition each)
N_WARM = 95  # PE warmup dummy matmuls

_CACHED = {}


def _level_of_node(i):
    return int(np.floor(np.log2(i + 1)))


def _build_program():
    import concourse.bacc as bacc
    import concourse.mybir as mybir
    import concourse.tile as tile
    import concourse.bass as bass

    f32 = mybir.dt.float32
    bf16 = mybir.dt.bfloat16
    fp8 = mybir.dt.float8e4

    nc = bacc.Bacc("TRN2", target_bir_lowering=False, debug=False,
                   num_devices=NCORES)

    # inputs (per-core shapes)
    xg = nc.dram_tensor("xg", [2, DEPTH, 128, NSH], f32, kind="ExternalInput")
    th = nc.dram_tensor("th", [2, 128, 15], f32, kind="ExternalInput")
    # l8[j, h, cc, p, kp, d, jj] = fp8 of (hi if h==0 else lo) of
    #   lut_perm[(2*kp+d)*256 + cc*128 + p, j*512 + jj]
    l8 = nc.dram_tensor("l8", [JSLABS, 2, 2, 128, 8, 2, 512], fp8,
                        kind="ExternalInput")
    out = nc.dram_tensor("out", [NCHUNK, 128, JSLABS, 512], bf16,
                         kind="ExternalOutput")

    gt = mybir.AluOpType.is_gt
    eq = mybir.AluOpType.is_equal
    add = mybir.AluOpType.add
    DR = mybir.MatmulPerfMode.DoubleRow

    with tile.TileContext(nc) as tc:
        # keep every pool open for the whole program: early closes let later
        # pools recycle SBUF ranges and inherit WAR waits on whole phases.
        with tc.tile_pool(name="enc", bufs=1) as encp, \
             tc.tile_pool(name="encwork", bufs=1) as wp, \
             tc.tile_pool(name="enctmp", bufs=1) as tp, \
             tc.tile_pool(name="lut", bufs=LUT_BUFS) as lutp, \
             tc.tile_pool(name="part", bufs=1) as pp, \
             tc.tile_pool(name="psum", bufs=8,
                          space=bass.MemorySpace.PSUM) as psp:

            # ---------------- PE warmup -----------------------------------
            wz = wp.tile([128, 2, 128], fp8, tag="wz")
            mz = wp.tile([128, 2, 512], fp8, tag="mz")
            nc.vector.memset(wz[:], 0.0)
            nc.vector.memset(mz[:], 0.0)
            pz = psp.tile([128, 512], f32, tag="ps", name="warm")
            for i in range(N_WARM):
                nc.tensor.matmul(pz[:], wz[:], mz[:],
                                 start=(i == 0), stop=(i == N_WARM - 1),
                                 perf_mode=DR)

            # ---------------- input DMAs (issue order matters) -------------
            tht = []
            xt = []
            lt = {}

            def load_lut(j, h, cc):
                t = lutp.tile([128, 8, 2, 512], fp8, tag="lut",
                              name=f"l{j}_{h}_{cc}")
                nc.sync.dma_start(t[:], l8[j, h, cc])
                lt[(j, h, cc)] = t

            for cc in range(2):
                t = wp.tile([128, 15], f32, tag=f"th{cc}")
                nc.sync.dma_start(t[:], th[cc])
                tht.append(t)
                row = []
                for l in range(DEPTH):
                    x = wp.tile([128, NSH], f32, tag=f"x{l}_{cc}",
                                name=f"x{l}_{cc}")
                    nc.sync.dma_start(x[:], xg[cc, l])
                    row.append(x)
                xt.append(row)
                if cc == 0:
                    # j0's slabs up front so the PE isn't lut-gated at start
                    load_lut(0, 0, 0)
                    load_lut(0, 1, 0)
            for j in range(1, JSLABS):
                load_lut(j, 0, 0)
                load_lut(j, 1, 0)
            for j in range(JSLABS):
                load_lut(j, 0, 1)
                load_lut(j, 1, 1)

            # one-hot tiles enc[(cc, off, kp)] for 256-wide n-slices.
            pieces = [(0, 0, 256), (0, 256, 256), (1, 0, 256), (1, 256, 256)]
            enc8 = {}
            for cc, off, w in pieces:
                for kp in range(8):
                    enc8[(cc, off, kp)] = encp.tile(
                        [128, 2, w], fp8, tag=f"e{cc}_{off}_{kp}",
                        name=f"e{cc}_{off}_{kp}")

            def encode_piece(cc, off, w):
                nsl = slice(off, off + w)
                B = [tp.tile([128, 256], bf16, tag=f"b{i}",
                             name=f"b{i}_{cc}{off}")[:, :w]
                     for i in range(15)]
                for i in range(15):
                    nc.vector.tensor_single_scalar(
                        B[i], xt[cc][_level_of_node(i)][:, nsl],
                        tht[cc][:, i:i + 1], gt)

                def mux(u, v, sel, tag):
                    # sel ? v : u, all values in {0,1}
                    t = tp.tile([128, 256], bf16, tag=tag,
                                name=f"mux_{tag}_{cc}{off}")[:, :w]
                    nc.vector.tensor_copy(t, u)
                    nc.vector.copy_predicated(t, sel, v)
                    return t

                b0 = B[0]
                b1 = mux(B[1], B[2], b0, "m1")
                m0 = mux(B[3], B[4], b1, "m20")
                m1 = mux(B[5], B[6], b1, "m21")
                b2 = mux(m0, m1, b0, "m2")
                c00 = mux(B[7], B[8], b2, "c00")
                c01 = mux(B[9], B[10], b2, "c01")
                c10 = mux(B[11], B[12], b2, "c10")
                c11 = mux(B[13], B[14], b2, "c11")
                d0 = mux(c00, c01, b1, "d0")
                d1 = mux(c10, c11, b1, "d1")
                b3 = mux(d0, d1, b0, "d")

                # idx = 8*b0 + 4*b1 + 2*b2 + b3 (small ints, exact in bf16)
                idx = tp.tile([128, 256], bf16, tag="idx",
                              name=f"idx{cc}{off}")[:, :w]
                nc.vector.tensor_scalar_mul(idx, b0, 2.0)
                nc.vector.tensor_add(idx, idx, b1)
                nc.vector.tensor_scalar_mul(idx, idx, 2.0)
                nc.vector.tensor_add(idx, idx, b2)
                nc.vector.tensor_scalar_mul(idx, idx, 2.0)
                nc.vector.tensor_add(idx, idx, b3)

                for k in range(K):
                    nc.vector.tensor_single_scalar(
                        enc8[(cc, off, k // 2)][:, k % 2, :], idx,
                        float(k), eq)

            for cc, off, w in pieces:
                encode_piece(cc, off, w)

            # weight slice for (cc, m): the enc piece covering m's n-range
            def wslice(cc, m, kp):
                for pcc, off, w in pieces:
                    if pcc == cc and off <= m * 128 < off + w:
                        o = m * 128 - off
                        return enc8[(cc, off, kp)][:, :, o:o + 128]
                raise KeyError

            # ---------------- matmul passes --------------------------------
            # every (cc, j, m) accumulation closes immediately (1-2 live PSUM
            # banks); cc0 closes to a bf16 partial, cc1 merges partial + PSUM
            # -> bf16 out on DVE.  cc0 order: m01 sweep over j0-3 first (only
            # needs the s0 encode piece), then the m23 backlog, then j4-7.
            part = {}

            def jm_tile(cc, j, m):
                ps = psp.tile([128, 512], f32, tag="ps",
                              name=f"ps{cc}_{j}_{m}")
                for kp in range(8):
                    w = wslice(cc, m, kp)
                    for h in range(2):
                        nc.tensor.matmul(
                            ps[:], w, lt[(j, h, cc)][:, kp, :, :],
                            start=(kp == 0 and h == 0),
                            stop=(kp == 7 and h == 1),
                            perf_mode=DR)
                if cc == 0:
                    pt = pp.tile([128, 512], bf16, tag=f"pt{j}_{m}",
                                 name=f"pt{j}_{m}")
                    part[(j, m)] = pt
                    nc.scalar.copy(pt[:], ps[:])
                else:
                    pt = part[(j, m)]
                    nc.vector.tensor_tensor(pt[:], pt[:], ps[:], add)
                    nc.sync.dma_start(out[m, :, j], pt[:])

            for j in range(4):
                for m in (0, 1):
                    jm_tile(0, j, m)
            for j in range(4):
                for m in (2, 3):
                    jm_tile(0, j, m)
            for j in range(4, JSLABS):
                for m in range(NCHUNK):
                    jm_tile(0, j, m)
            for j in range(JSLABS):
                for m in range(NCHUNK):
                    jm_tile(1, j, m)

    nc.compile()
    return nc


_BASE_TREE = np.array([
    [-1,-1,0,-1,0,0,0,-1,0,0,0,0,0,0,0],[-1,-1,0,-1,0,0,0,1,0,0,0,0,0,0,0],
    [-1,-1,0,1,0,0,0,0,-1,0,0,0,0,0,0],[-1,-1,0,1,0,0,0,0,1,0,0,0,0,0,0],
    [-1,1,0,0,-1,0,0,0,0,-1,0,0,0,0,0],[-1,1,0,0,-1,0,0,0,0,1,0,0,0,0,0],
    [-1,1,0,0,1,0,0,0,0,0,-1,0,0,0,0],[-1,1,0,0,1,0,0,0,0,0,1,0,0,0,0],
    [1,0,-1,0,0,-1,0,0,0,0,0,-1,0,0,0],[1,0,-1,0,0,-1,0,0,0,0,0,1,0,0,0],
    [1,0,-1,0,0,1,0,0,0,0,0,0,-1,0,0],[1,0,-1,0,0,1,0,0,0,0,0,0,1,0,0],
    [1,0,1,0,0,0,-1,0,0,0,0,0,0,-1,0],[1,0,1,0,0,0,-1,0,0,0,0,0,0,1,0],
    [1,0,1,0,0,0,1,0,0,0,0,0,0,0,-1],[1,0,1,0,0,0,1,0,0,0,0,0,0,0,1]],
    dtype=np.float32)


def _reference_structure_ok(selection_matrix, tree_des_mat):
    sm = np.asarray(selection_matrix)
    td = np.asarray(tree_des_mat)
    if sm.shape != (C * (K - 1), C * DEPTH) or td.shape != (C * K, C * (K - 1)):
        return False
    base_sel = np.zeros((K - 1, DEPTH), dtype=np.float32)
    base_sel[0, 0] = 1.0
    for i in range(1, K - 1):
        base_sel[i, int(np.log2(i + 1))] = 1.0
    exp_sm = np.zeros_like(sm)
    exp_td = np.ones_like(td)
    for i in range(C):
        exp_sm[i * (K - 1):(i + 1) * (K - 1), i * DEPTH:(i + 1) * DEPTH] = base_sel
        exp_td[i * K:(i + 1) * K, i * (K - 1):(i + 1) * (K - 1)] = _BASE_TREE
    return np.array_equal(sm, exp_sm) and np.array_equal(td, exp_td)


def _numpy_fallback(inputMatrix, dims, selection_matrix, thresholds,
                    tree_des_mat, lut):
    """Faithful numpy replication of the reference forward pass (slow)."""
    x = np.asarray(inputMatrix, np.float32)
    n = x.shape[0]
    c = lut.shape[1]
    chosen = x[:, np.asarray(dims).astype(np.int64)]
    subtracted = (np.asarray(selection_matrix, np.float32) @ chosen.T
                  - np.asarray(thresholds, np.float32))
    sign = np.sign(subtracted).astype(np.float32)
    tree_result = (np.asarray(tree_des_mat, np.float32) @ sign).T.reshape(n, c, K)
    index = np.argmax(tree_result, axis=2)
    onehot = np.eye(K, dtype=np.float32)[index]  # (n, c, K)
    lutm = np.asarray(lut, np.float32).transpose(1, 2, 0).reshape(c * K, -1)
    return (onehot.reshape(n, c * K) @ lutm).astype(np.float32)


def kernel(inputMatrix, dims, selection_matrix, thresholds, tree_des_mat, lut):
    inputMatrix = np.ascontiguousarray(np.asarray(inputMatrix, dtype=np.float32))
    dims_i = np.asarray(dims).astype(np.int64)
    thresholds = np.asarray(thresholds, dtype=np.float32)
    lut = np.asarray(lut, dtype=np.float32)

    if not _reference_structure_ok(selection_matrix, tree_des_mat):
        return _numpy_fallback(inputMatrix, dims_i, selection_matrix,
                               thresholds, tree_des_mat, lut)

    # ---- host prep ----
    chosen = inputMatrix[:, dims_i]  # (N, C*DEPTH)
    th3 = np.ascontiguousarray(thresholds.reshape(C, K - 1).reshape(2, 128, 15))

    # lut_perm[k*256+c, j] = lut[j, c, k]; fp8 hi + fp8 lo residual
    lut_perm = np.ascontiguousarray(
        lut.transpose(2, 1, 0).reshape(C * K, OUT_FEATURES))
    lut_hi = lut_perm.astype(ml_dtypes.float8_e4m3)
    lut_lo = (lut_perm - lut_hi.astype(np.float32)).astype(ml_dtypes.float8_e4m3)

    def dev_layout(a):
        # (4096 ck, 4096 j) -> [j, cc, p, kp, d, jj], ck = (2kp+d)*256+cc*128+p
        return a.reshape(8, 2, 2, 128, JSLABS, 512).transpose(4, 2, 3, 0, 1, 5)

    l8_np = np.ascontiguousarray(
        np.stack([dev_layout(lut_hi), dev_layout(lut_lo)], axis=1))

    from concourse.bass_utils import run_bass_kernel_spmd

    if "nc" not in _CACHED:
        _CACHED["nc"] = _build_program()
    nc = _CACHED["nc"]

    in_maps = []
    for g in range(NCORES):
        ch = chosen[g * NSH:(g + 1) * NSH].reshape(NSH, 2, 128, DEPTH)
        xg_np = np.ascontiguousarray(ch.transpose(1, 3, 2, 0))  # [cc, l, p, n]
        in_maps.append({"xg": xg_np, "th": th3, "l8": l8_np})

    res = run_bass_kernel_spmd(nc, in_maps, list(range(NCORES)))
    out = np.concatenate(
        [np.asarray(res.results[g]["out"]).astype(np.float32)
         .reshape(NSH, OUT_FEATURES) for g in range(NCORES)], axis=0)
    return out
